# revision 1
# baseline (speedup 1.0000x reference)
"""Trainium2 Bass kernel for gated multi-head attention (nn_Attention_71751723647784).

Reference computation (B=1, Q=K=2048, CQ=CK=CV=128, H=8, CH=32, HD=256):
    q = (q_x @ Wq)/sqrt(CH); k = kv_x @ Wk; v = kv_x @ Wv           (per-head CH=32)
    a = softmax(q k^T + bias + distance.transpose(0,3,1,2), axis=-1)
    o = (a @ v) * sigmoid(q_x @ Wg + bg);  out = o @ Wo + bo

Sharding: rows of Q across the 8 cores (256 query rows per core). Every input
byte is read exactly once (bias is shared across heads, so head-sharding would
re-read it 8x); no collectives are needed -- each core produces 256 output rows.
"""

import math
import numpy as np

B, Q, KS = 1, 2048, 2048
CQ = 128
H, CH = 8, 32
HD = H * CH  # 256
NCORES = 8
QL = Q // NCORES       # 256 query rows per core
QT = 128               # q-tile (partition dim)
NQT = QL // QT         # 2 q-tiles per core
KC = 512               # k-chunk for score matmuls (one PSUM bank)
NKC = KS // KC         # 4 chunks
SCALE = 1.0 / math.sqrt(CH)
NDVE = 2  # heads per 4-group whose distance-add runs on DVE instead of PE

_CACHE = {}


def build_nc():
    from concourse import bacc
    import concourse.tile as tile
    import concourse.bass as bass
    import concourse.mybir as mybir
    from concourse.masks import make_identity

    f32 = mybir.dt.float32
    bf16 = mybir.dt.bfloat16
    AF = mybir.ActivationFunctionType
    ALU = mybir.AluOpType

    nc = bacc.Bacc("TRN2", target_bir_lowering=False, debug=False)

    q_x = nc.dram_tensor("q_x", (QL, CQ), f32, kind="ExternalInput").ap()
    kv_x = nc.dram_tensor("kv_x", (KS, CQ), f32, kind="ExternalInput").ap()
    bias = nc.dram_tensor("bias", (QL, KS), f32, kind="ExternalInput").ap()
    dist = nc.dram_tensor("distance", (H, QL, KS), f32, kind="ExternalInput").ap()
    Wq = nc.dram_tensor("Wq", (CQ, HD), f32, kind="ExternalInput").ap()
    Wk = nc.dram_tensor("Wk", (CQ, HD), f32, kind="ExternalInput").ap()
    Wv = nc.dram_tensor("Wv", (CQ, HD), f32, kind="ExternalInput").ap()
    Wg = nc.dram_tensor("Wg", (CQ, HD), f32, kind="ExternalInput").ap()
    bg = nc.dram_tensor("bg", (HD,), f32, kind="ExternalInput").ap()
    Wo = nc.dram_tensor("Wo", (HD, CQ), f32, kind="ExternalInput").ap()
    bo = nc.dram_tensor("bo", (CQ,), f32, kind="ExternalInput").ap()
    out = nc.dram_tensor("out", (QL, CQ), f32, kind="ExternalOutput").ap()

    with tile.TileContext(nc) as tc:
        with (
            tc.tile_pool(name="const", bufs=1) as constp,
            tc.tile_pool(name="wts", bufs=1) as wtp,
            tc.tile_pool(name="proj", bufs=1) as projp,
            tc.tile_pool(name="dist", bufs=4) as distp,
            tc.tile_pool(name="scores", bufs=2) as scp,
            tc.tile_pool(name="scoreonly", bufs=2) as sc2p,
            tc.tile_pool(name="e", bufs=2) as ep,
            tc.tile_pool(name="eT", bufs=5) as etp,
            tc.tile_pool(name="small", bufs=2) as smp,
            tc.tile_pool(name="psA", bufs=2, space="PSUM") as psA,
            tc.tile_pool(name="psO", bufs=4, space="PSUM") as psO,
        ):
            # ---- constants ----
            ident_bf = constp.tile([128, 128], bf16)
            make_identity(nc, ident_bf[:])
            ones_bf = constp.tile([1, QL], bf16)
            nc.gpsimd.memset(ones_bf[:], 1.0)
            zeros_bf = constp.tile([1, 128], bf16)
            nc.gpsimd.memset(zeros_bf[:], 0.0)

            # ~4us of dummy matmuls while initial DMAs land: trips the PE HAM
            # activity monitor so real matmuls start at 2.4 GHz, not 1.2.
            wps = psA.tile([128, 512], f32, tag="psA", name="warm")
            for i in range(10):
                nc.tensor.matmul(wps[:, 0:128], lhsT=ident_bf[:],
                                 rhs=ident_bf[:], start=True, stop=True)

            # ---- weights: plain f32 HWDGE loads (the SWDGE cast-DMA path
            # measures ~20 GB/s -- never bulk-load through it), cast on DVE --
            wf = scp.tile([128, 4 * HD + 2 * 128 + 128], f32, tag="stage", name="wf")
            nc.scalar.dma_start(wf[:, 0:HD], Wq)
            nc.scalar.dma_start(wf[:, HD:2 * HD], Wk)
            nc.scalar.dma_start(wf[:, 2 * HD:3 * HD], Wv)
            nc.scalar.dma_start(wf[:, 3 * HD:4 * HD], Wg)
            wo_v = Wo.rearrange("(g p) c -> p g c", p=128)
            nc.scalar.dma_start(wf[:, 4 * HD:4 * HD + 128], wo_v[:, 0, :])
            nc.scalar.dma_start(wf[:, 4 * HD + 128:4 * HD + 256], wo_v[:, 1, :])
            nc.scalar.dma_start(wf[0:1, 4 * HD + 256:4 * HD + 256 + 128],
                              bo.rearrange("(a c) -> a c", a=1))
            wq_sb = wtp.tile([128, HD], bf16)
            wk_sb = wtp.tile([128, HD], bf16)
            wv_sb = wtp.tile([128, HD], bf16)
            wg_sb = wtp.tile([128, HD], bf16)
            wo_sb = wtp.tile([128, 2, 128], bf16)
            bo_sb = wtp.tile([1, 128], bf16)
            bg_sb = wtp.tile([128, 2], f32)
            nc.vector.tensor_copy(wq_sb[:], wf[:, 0:HD])
            nc.vector.tensor_copy(wk_sb[:], wf[:, HD:2 * HD])
            nc.vector.tensor_copy(wv_sb[:], wf[:, 2 * HD:3 * HD])
            nc.vector.tensor_copy(wg_sb[:], wf[:, 3 * HD:4 * HD])
            for g_ in range(2):
                nc.vector.tensor_copy(
                    wo_sb[:, g_, :],
                    wf[:, 4 * HD + 128 * g_:4 * HD + 128 * (g_ + 1)])
            nc.vector.tensor_copy(bo_sb[:], wf[0:1, 4 * HD + 256:4 * HD + 384])
            nc.scalar.dma_start(bg_sb[:], bg.rearrange("(g p) -> p g", p=128))

            # ---- activations: f32 loads + DVE cast, then transpose on PE ----
            qx_f = scp.tile([128, NQT, 128], f32, tag="stage", name="qx_f")
            nc.scalar.dma_start(qx_f[:], q_x.rearrange("(a p) c -> p a c", p=128))
            kvx_f = scp.tile([128, 16, 128], f32, tag="stage", name="kvx_f")
            nc.scalar.dma_start(kvx_f[:], kv_x.rearrange("(a p) c -> p a c", p=128))
            qx_bf = projp.tile([128, NQT, 128], bf16)
            nc.vector.tensor_copy(qx_bf[:], qx_f[:])
            kvx_bf = projp.tile([128, 16, 128], bf16)
            nc.vector.tensor_copy(kvx_bf[:], kvx_f[:])

            qxT = projp.tile([128, QL], bf16)      # [CQ, QL]
            for i in range(NQT):
                ps = psA.tile([128, 128], bf16, tag="psA")
                nc.tensor.transpose(ps[:], qx_bf[:, i, :], ident_bf[:])
                nc.vector.tensor_copy(qxT[:, i * 128:(i + 1) * 128], ps[:])
            kvxT = projp.tile([128, KS], bf16)     # [CQ, K]
            for i in range(16):
                ps = psA.tile([128, 128], bf16, tag="psA")
                nc.tensor.transpose(ps[:], kvx_bf[:, i, :], ident_bf[:])
                nc.vector.tensor_copy(kvxT[:, i * 128:(i + 1) * 128], ps[:])

            # ---- projections ----
            # qT[hd, q] (scaled by 1/sqrt(CH)), kT[hd, k], per hd-half g
            qT = [projp.tile([128, QL], bf16, tag=f"qT{g}", name=f"qT{g}") for g in range(2)]
            kT = [projp.tile([128, KS], bf16, tag=f"kT{g}", name=f"kT{g}") for g in range(2)]
            for g in range(2):
                ps = psA.tile([128, 256], f32, tag="psA")
                nc.tensor.matmul(ps[:], lhsT=wq_sb[:, g * 128:(g + 1) * 128],
                                 rhs=qxT[:], start=True, stop=True)
                nc.scalar.activation(qT[g][:], ps[:], AF.Copy, scale=SCALE)
                for c in range(NKC):
                    ps2 = psA.tile([128, KC], f32, tag="psA")
                    nc.tensor.matmul(ps2[:], lhsT=wk_sb[:, g * 128:(g + 1) * 128],
                                     rhs=kvxT[:, c * KC:(c + 1) * KC],
                                     start=True, stop=True)
                    nc.scalar.copy(kT[g][:, c * KC:(c + 1) * KC], ps2[:])
            # v[k, hd] in 16 k-tiles
            v_sb = projp.tile([128, 16, HD], bf16)
            for kt in range(16):
                ps = psA.tile([128, HD], f32, tag="psA")
                nc.tensor.matmul(ps[:], lhsT=kvxT[:, kt * 128:(kt + 1) * 128],
                                 rhs=wv_sb[:], start=True, stop=True)
                nc.vector.tensor_copy(v_sb[:, kt, :], ps[:])
            # gT[hd, q] = sigmoid(Wg^T qx + bg), full width per hd-half
            gTf = [projp.tile([128, QL], bf16, tag=f"gTf{g}", name=f"gTf{g}")
                   for g in range(2)]
            for g in range(2):
                ps = psA.tile([128, QL], f32, tag="psA")
                nc.tensor.matmul(ps[:], lhsT=wg_sb[:, g * 128:(g + 1) * 128],
                                 rhs=qxT[:], start=True, stop=True)
                nc.scalar.activation(gTf[g][:], ps[:], AF.Sigmoid,
                                     bias=bg_sb[:, g:g + 1])

            # second HAM warm burst anchored on kT (runs just before the
            # first scores; keeps the PE at 2.4 GHz into the main loop)
            wps2 = psA.tile([128, 1024], f32, tag="psA", name="warm2")
            for i in range(8):
                nc.tensor.matmul(wps2[:, 0:512], lhsT=ident_bf[:],
                                 rhs=kT[0][:, 0:512], start=True, stop=True)

            # second HAM warm burst anchored on kT: lands right before the
            # first scores so the PE enters the main loop at 2.4 GHz
            wps2 = psA.tile([128, 1024], f32, tag="psA", name="warm2")
            for i in range(12):
                nc.tensor.matmul(wps2[:, 0:512], lhsT=ident_bf[:],
                                 rhs=kT[0][:, 0:512], start=True, stop=True)

            # ---- main attention loop ----
            # distance is pre-sliced h-major on the host (the sharding hint's
            # "distance sliced on H"), so every load and operand is contiguous
            dview = dist.rearrange("h (a p) k -> h a p k", p=128)
            bias_bf = []
            for qt in range(NQT):
                bf_ = scp.tile([128, KS], f32, tag="stage", name=f"biasf{qt}")
                nc.scalar.dma_start(
                    bf_[:], bias.rearrange("(a p) k -> a p k", p=128)[qt])
                bb = distp.tile([128, KS], bf16, tag=f"bias{qt}",
                                name=f"bias{qt}")
                nc.vector.tensor_copy(bb[:], bf_[:])
                bias_bf.append(bb)

            gos = []
            eTs = {}
            dpart = smp.tile([128, 4 * H], f32, tag="dpartA")
            recipA = smp.tile([128, 2 * H], f32, tag="recipA")
            for h in range(H):
                g, hl = h // 4, h % 4
                dve_head = hl >= 4 - NDVE
                et = etp.tile([128, 16, QL], bf16, tag="eT")
                for qt in range(NQT):
                    df = sc2p.tile([128, KS], f32, tag="dfstage",
                                   name=f"df{h}{qt}")
                    nc.sync.dma_start(df[:], dview[h, qt])
                    dbf = distp.tile([128, KS], bf16, tag="dbf")
                    nc.vector.tensor_copy(dbf[:], df[:])
                    e_sb = ep.tile([128, KS], bf16, tag="e")
                    if dve_head:
                        score = sc2p.tile([128, KS], f32, tag="score")
                    for s in range(2):
                        ps = psA.tile([128, 1024], f32, tag="psA")
                        ssl = slice(s * 1024, (s + 1) * 1024)
                        if not dve_head:
                            for c in range(2):
                                ksl = slice((2 * s + c) * KC,
                                            (2 * s + c + 1) * KC)
                                nc.tensor.matmul(ps[:, c * KC:(c + 1) * KC],
                                                 lhsT=ident_bf[:],
                                                 rhs=dbf[:, ksl],
                                                 start=True, stop=False)
                        for c in range(2):
                            ksl = slice((2 * s + c) * KC, (2 * s + c + 1) * KC)
                            nc.tensor.matmul(ps[:, c * KC:(c + 1) * KC],
                                             lhsT=ident_bf[:],
                                             rhs=bias_bf[qt][:, ksl],
                                             start=dve_head, stop=False)
                        for c in range(2):
                            ksl = slice((2 * s + c) * KC, (2 * s + c + 1) * KC)
                            nc.tensor.matmul(
                                ps[:, c * KC:(c + 1) * KC],
                                lhsT=qT[g][32 * hl:32 * hl + 32,
                                           qt * 128:(qt + 1) * 128],
                                rhs=kT[g][32 * hl:32 * hl + 32, ksl],
                                start=False, stop=True,
                                tile_position=(32 * hl, 0))
                        if dve_head:
                            nc.vector.scalar_tensor_tensor(
                                out=score[:, ssl], in0=ps[:], scalar=1.0,
                                in1=dbf[:, ssl], op0=ALU.mult, op1=ALU.add)
                            nc.scalar.activation(
                                e_sb[:, ssl], score[:, ssl], AF.Exp,
                                accum_out=dpart[:, 4 * qt + 2 * s
                                                :4 * qt + 2 * s + 1])
                        else:
                            nc.scalar.activation(
                                e_sb[:, ssl], ps[:], AF.Exp,
                                accum_out=dpart[:, 4 * qt + 2 * s
                                                :4 * qt + 2 * s + 1])
                    nc.vector.tensor_add(
                        recipA[:, 2 * h + qt:2 * h + qt + 1],
                        dpart[:, 4 * qt:4 * qt + 1],
                        dpart[:, 4 * qt + 2:4 * qt + 3])
                    nc.vector.reciprocal(recipA[:, 2 * h + qt:2 * h + qt + 1],
                                         recipA[:, 2 * h + qt:2 * h + qt + 1])
                    e_n = ep.tile([128, KS], bf16, tag="en")
                    nc.vector.tensor_scalar_mul(
                        e_n[:], e_sb[:], recipA[:, 2 * h + qt:2 * h + qt + 1])
                    # all transposes on the sync queue: the scalar queue
                    # carries the critical exp chain in the main phase
                    nc.sync.dma_start_transpose(
                        et[:, :, qt * 128:(qt + 1) * 128], e_n[:])
                eTs[h] = et

                if hl == 3:
                    # AV: one PSUM bank per head (concurrent accumulation
                    # streams; bank-clear on start can never hit a sibling)
                    psos = []
                    for hl2 in range(4):
                        p_ = psO.tile([128, QL], f32, tag="psO",
                                      name=f"pso{g}{hl2}")
                        psos.append(p_)
                    for kt in range(16):
                        for hl2 in range(4):
                            h2 = g * 4 + hl2
                            nc.tensor.matmul(
                                psos[hl2][32 * hl2:32 * hl2 + 32, :],
                                lhsT=v_sb[:, kt, 32 * h2:32 * h2 + 32],
                                rhs=eTs[h2][:, kt, :],
                                start=(kt == 0), stop=(kt == 15),
                                tile_position=(0, 32 * hl2))
                    go = smp.tile([128, QL], bf16, tag="go")
                    for hl2 in range(4):
                        sl = slice(32 * hl2, 32 * hl2 + 32)
                        nc.vector.tensor_mul(go[sl, :], psos[hl2][sl, :],
                                             gTf[g][sl, :])
                    gos.append(go)

            # final projection: out[q, co] = sum_hd go[hd, q] * Wo[hd, co] + bo
            for qt in range(NQT):
                qsl = slice(qt * 128, (qt + 1) * 128)
                psout = psA.tile([128, 128], f32, tag="psA")
                nc.tensor.matmul(psout[:], lhsT=gos[0][:, qsl],
                                 rhs=wo_sb[:, 0, :], start=True, stop=False)
                nc.tensor.matmul(psout[:], lhsT=gos[1][:, qsl],
                                 rhs=wo_sb[:, 1, :], start=False, stop=False)
                nc.tensor.matmul(psout[:], lhsT=ones_bf[:, 0:128], rhs=bo_sb[:],
                                 start=False, stop=True)
                out_sb = smp.tile([128, 128], f32, tag="out")
                nc.vector.tensor_copy(out_sb[:], psout[:])
                nc.sync.dma_start(
                    out.rearrange("(a p) c -> a p c", p=128)[qt], out_sb[:])

    nc.compile()
    return nc


def _get_nc():
    if "nc" not in _CACHE:
        _CACHE["nc"] = build_nc()
    return _CACHE["nc"]


def make_in_maps(q_x, kv_x, bias, distance, Wq, Wk, Wv, Wg, bg, Wo, bo):
    com = {
        "kv_x": np.ascontiguousarray(kv_x[0]),
        "Wq": np.ascontiguousarray(Wq), "Wk": np.ascontiguousarray(Wk),
        "Wv": np.ascontiguousarray(Wv), "Wg": np.ascontiguousarray(Wg),
        "bg": np.ascontiguousarray(bg), "Wo": np.ascontiguousarray(Wo),
        "bo": np.ascontiguousarray(bo),
    }
    maps = []
    for i in range(NCORES):
        s = slice(i * QL, (i + 1) * QL)
        m = dict(com)
        m["q_x"] = np.ascontiguousarray(q_x[0, s])
        m["bias"] = np.ascontiguousarray(bias[0, 0, s])
        m["distance"] = np.ascontiguousarray(np.transpose(distance[0, s], (2, 0, 1)))
        maps.append(m)
    return maps


def kernel(q_x, kv_x, bias, distance, Wq, Wk, Wv, Wg, bg, Wo, bo, trace=False):
    from concourse.bass_utils import run_bass_kernel_spmd

    nc = _get_nc()
    in_maps = make_in_maps(np.asarray(q_x, np.float32), np.asarray(kv_x, np.float32),
                           np.asarray(bias, np.float32),
                           np.asarray(distance, np.float32),
                           np.asarray(Wq, np.float32), np.asarray(Wk, np.float32),
                           np.asarray(Wv, np.float32), np.asarray(Wg, np.float32),
                           np.asarray(bg, np.float32), np.asarray(Wo, np.float32),
                           np.asarray(bo, np.float32))
    res = run_bass_kernel_spmd(nc, in_maps, core_ids=list(range(NCORES)),
                               trace=trace)
    _CACHE["last_result"] = res
    out = np.concatenate([res.results[i]["out"] for i in range(NCORES)], axis=0)
    return out.reshape(B, Q, CQ).astype(np.float32)



# revision 10
# speedup vs baseline: 1.2938x; 1.2938x over previous
"""Trainium2 Bass kernel for gated multi-head attention (nn_Attention_71751723647784).

Reference computation (B=1, Q=K=2048, CQ=CK=CV=128, H=8, CH=32, HD=256):
    q = (q_x @ Wq)/sqrt(CH); k = kv_x @ Wk; v = kv_x @ Wv           (per-head CH=32)
    a = softmax(q k^T + bias + distance.transpose(0,3,1,2), axis=-1)
    o = (a @ v) * sigmoid(q_x @ Wg + bg);  out = o @ Wo + bo

Sharding: rows of Q across the 8 cores (256 query rows per core); every input
byte read exactly once (bias is shared across heads).

v2 design (k-major scores):
  - Bulk inputs are cast/laid out in bf16 on the host: distance arrives as
    [H, 128p, 16kt*256q] (k = kt*128+p), bias as [128p, 16kt*256q], q_x/kv_x
    pre-transposed to [c, q]/[c, k], Wq pre-scaled by 1/sqrt(CH).  Halves DMA
    bytes and removes every on-chip cast / PE input transpose of v1.
  - Scores are built k-major (k on partitions): sc[k,q] = qk (PE matmul)
    + (bias+dist) where bias+dist is one DVE bf16 add (2x mode) merged into
    PSUM with one PE identity-matmul.  exp on ACT reads PSUM directly; no
    max-subtraction needed (scores are O(6), bf16/f32 range is ample).
  - Softmax normalisation is deferred past the AV matmul: v carries an
    appended ones-column, so row 32 of each head's AV output is the softmax
    denominator for free.  recip via reciprocal_approx_fast, broadcast over
    the 32 ch rows with a tiny PE outer-product, folded into the gating mults.
  - No e-transposes (33us of DMA_TRANSPOSE in v1) and no separate
    normalisation pass over e (12us of DVE in v1).
"""

import math
import numpy as np

B, Q, KS = 1, 2048, 2048
CQ = 128
H, CH = 8, 32
HD = H * CH  # 256
NCORES = 8
QL = Q // NCORES       # 256 query rows per core
NKT = 16               # k tiles of 128
KW = NKT * QL          # 4096 score elements per partition per head
SCALE = 1.0 / math.sqrt(CH)
# exp chunks in k-tiles: 6+6+4 tiles -> ACT FD 1536/1536/1024
CHUNKS = [(0, 6), (6, 6), (12, 4)]

_CACHE = {}


def build_nc():
    from concourse import bacc
    import concourse.tile as tile
    import concourse.mybir as mybir
    from concourse.masks import make_identity

    f32 = mybir.dt.float32
    bf16 = mybir.dt.bfloat16
    AF = mybir.ActivationFunctionType

    nc = bacc.Bacc("TRN2", target_bir_lowering=False, debug=False)

    dist = nc.dram_tensor("distance", (H, 128, KW), bf16, kind="ExternalInput").ap()
    biasT = nc.dram_tensor("bias", (128, KW), bf16, kind="ExternalInput").ap()
    qxT = nc.dram_tensor("q_x", (CQ, QL), bf16, kind="ExternalInput").ap()
    kvxT = nc.dram_tensor("kv_x", (CQ, KS), bf16, kind="ExternalInput").ap()
    Wq = nc.dram_tensor("Wq", (CQ, HD), bf16, kind="ExternalInput").ap()  # pre-scaled
    Wk = nc.dram_tensor("Wk", (CQ, HD), bf16, kind="ExternalInput").ap()
    Wv = nc.dram_tensor("Wv", (CQ, HD), bf16, kind="ExternalInput").ap()
    Wg = nc.dram_tensor("Wg", (CQ, HD), bf16, kind="ExternalInput").ap()
    bgr = nc.dram_tensor("bg", (1, HD), bf16, kind="ExternalInput").ap()
    Wo = nc.dram_tensor("Wo", (32, H, CQ), bf16, kind="ExternalInput").ap()
    bo = nc.dram_tensor("bo", (1, CQ), bf16, kind="ExternalInput").ap()
    out = nc.dram_tensor("out", (QL, CQ), f32, kind="ExternalOutput").ap()

    with tile.TileContext(nc) as tc:
        with (
            tc.tile_pool(name="const", bufs=1) as constp,
            tc.tile_pool(name="wts", bufs=1) as wtp,
            tc.tile_pool(name="proj", bufs=1) as projp,
            tc.tile_pool(name="dist", bufs=3) as distp,
            tc.tile_pool(name="bd", bufs=2) as bdp,
            tc.tile_pool(name="e", bufs=2) as ep,
            tc.tile_pool(name="small", bufs=3) as smp,
            tc.tile_pool(name="psSC", bufs=2, space="PSUM") as psSC,
            tc.tile_pool(name="psX", bufs=2, space="PSUM") as psX,
        ):
            # ---- constants ----
            ident_bf = constp.tile([128, 128], bf16)
            make_identity(nc, ident_bf[:])
            ones_row = constp.tile([1, QL], bf16)
            nc.gpsimd.memset(ones_row[:], 1.0)
            ones33 = constp.tile([33, 32], bf16)
            nc.gpsimd.memset(ones33[:], 1.0)

            # ---- weight / input loads (scalar HWDGE queue) ----
            wq_sb = wtp.tile([128, HD], bf16)
            wk_sb = wtp.tile([128, HD], bf16)
            wv_sb = wtp.tile([128, HD], bf16)
            wg_sb = wtp.tile([128, HD], bf16)
            wo_sb = wtp.tile([32, H, 128], bf16)
            bo_sb = wtp.tile([1, 128], bf16)
            bg_sb = wtp.tile([1, HD], bf16)
            qxT_sb = wtp.tile([128, QL], bf16)
            kvxT_sb = wtp.tile([128, KS], bf16)
            biasT_sb = wtp.tile([128, KW], bf16)
            nc.scalar.dma_start(qxT_sb[:], qxT)
            nc.scalar.dma_start(wg_sb[:], Wg)
            nc.scalar.dma_start(bg_sb[:], bgr)
            nc.scalar.dma_start(wq_sb[:], Wq)
            nc.scalar.dma_start(wk_sb[:], Wk)
            nc.scalar.dma_start(wv_sb[:], Wv)
            nc.scalar.dma_start(wo_sb[:], Wo)
            nc.scalar.dma_start(bo_sb[:], bo)
            nc.scalar.dma_start(kvxT_sb[:], kvxT)
            nc.scalar.dma_start(biasT_sb[:], biasT)

            # ---- PE warm-up: trip the HAM activity monitor early so the
            # main loop runs at 2.4 GHz (~5us of back-to-back matmuls).
            wps = psSC.tile([128, 1536], f32, tag="sc", name="warm")
            for i in range(24):
                nc.tensor.matmul(wps[:, 0:128], lhsT=ident_bf[:],
                                 rhs=ident_bf[:], start=True, stop=True)

            # ---- projections ----
            # gating first (sigmoid's ACT table loads before exp's).
            # gT8[c, h, q] = sigmoid(Wg_h^T qxT + bg_h), per-head at rows 0-31
            gT8 = projp.tile([32, H, QL], bf16)
            psg1 = psSC.tile([128, 1536], f32, tag="sc", name="psg1")
            psg2 = psX.tile([128, 512], f32, tag="px", name="psg2")
            for h in range(H):
                dst = psg1[0:32, h * QL:(h + 1) * QL] if h < 6 else \
                      psg2[0:32, (h - 6) * QL:(h - 5) * QL]
                nc.tensor.matmul(dst, lhsT=wg_sb[:, 32 * h:32 * h + 32],
                                 rhs=qxT_sb[:], start=True, stop=False)
                nc.tensor.matmul(dst, lhsT=bg_sb[:, 32 * h:32 * h + 32],
                                 rhs=ones_row[:], start=False, stop=True)
            nc.scalar.activation(
                gT8[:, 0:6, :].rearrange("c a q -> c (a q)"),
                psg1[0:32, 0:6 * QL], AF.Sigmoid)
            nc.scalar.activation(
                gT8[:, 6:8, :].rearrange("c a q -> c (a q)"),
                psg2[0:32, 0:2 * QL], AF.Sigmoid)
            # force the exp table load now, during the DMA window
            je = smp.tile([128, 2], bf16, tag="junk")
            nc.scalar.activation(je[:], qxT_sb[:, 0:2], AF.Exp)

            # qT[g][hd-half, q] (Wq pre-scaled on host)
            qT = []
            for g in range(2):
                psq = psX.tile([128, 512], f32, tag="px", name=f"psq{g}")
                nc.tensor.matmul(psq[:, 0:QL], lhsT=wq_sb[:, g * 128:(g + 1) * 128],
                                 rhs=qxT_sb[:], start=True, stop=True)
                qt = projp.tile([128, QL], bf16, tag=f"qT{g}", name=f"qT{g}")
                nc.vector.tensor_copy(qt[:], psq[:, 0:QL])
                qT.append(qt)
            # kT[g][hd-half, k] full width
            kT = []
            for g in range(2):
                kt_ = projp.tile([128, KS], bf16, tag=f"kT{g}", name=f"kT{g}")
                for c in range(2):
                    psk = psSC.tile([128, 1536], f32, tag="sc", name=f"psk{g}{c}")
                    for j in range(2):
                        nc.tensor.matmul(
                            psk[:, j * 512:(j + 1) * 512],
                            lhsT=wk_sb[:, g * 128:(g + 1) * 128],
                            rhs=kvxT_sb[:, c * 1024 + j * 512:c * 1024 + (j + 1) * 512],
                            start=True, stop=True)
                    nc.vector.tensor_copy(kt_[:, c * 1024:(c + 1) * 1024],
                                          psk[:, 0:1024])
                kT.append(kt_)
            # v1[k, kt, h, 0:32]=v, [.., 32]=1.0 (denominator column)
            v1 = projp.tile([128, NKT, H, 36], bf16)
            nc.gpsimd.memset(v1[:, :, :, 32:33], 1.0)
            for kt in range(NKT):
                psv = psX.tile([128, 512], f32, tag="px", name=f"psv{kt}")
                nc.tensor.matmul(psv[:, 0:HD],
                                 lhsT=kvxT_sb[:, kt * 128:(kt + 1) * 128],
                                 rhs=wv_sb[:], start=True, stop=True)
                nc.vector.tensor_copy(
                    v1[:, kt, :, 0:32],
                    psv[:, 0:HD].rearrange("p (h c) -> p h c", h=H))

            # ---- main loop over heads ----
            go_all = projp.tile([128, H, QL], bf16)  # rows 0-31 live
            for h in range(H):
                g, hl = h // 4, h % 4
                d_sb = distp.tile([128, KW], bf16, tag="dist")
                nc.sync.dma_start(d_sb[:], dist[h])
                bd = bdp.tile([128, KW], bf16, tag="bd")
                nc.vector.tensor_add(bd[:], d_sb[:], biasT_sb[:])
                e_sb = ep.tile([128, KW], bf16, tag="e")
                for (kt0, nkt) in CHUNKS:
                    sc = psSC.tile([128, 1536], f32, tag="sc")
                    for j in range(nkt):
                        kt = kt0 + j
                        sl = slice(j * QL, (j + 1) * QL)
                        nc.tensor.matmul(
                            sc[:, sl],
                            lhsT=kT[g][32 * hl:32 * hl + 32, kt * 128:(kt + 1) * 128],
                            rhs=qT[g][32 * hl:32 * hl + 32, :],
                            start=True, stop=False, tile_position=(32 * hl, 0))
                        nc.tensor.matmul(
                            sc[:, sl], lhsT=ident_bf[:],
                            rhs=bd[:, kt * QL:(kt + 1) * QL],
                            start=False, stop=True)
                    nc.scalar.activation(
                        e_sb[:, kt0 * QL:(kt0 + nkt) * QL],
                        sc[:, 0:nkt * QL], AF.Exp)
                # AV with ones-column: rows 0-31 data, row 32 = denominator
                av = psX.tile([128, 512], f32, tag="px", name=f"av{h}")
                avs = av[0:33, 0:QL]
                for kt in range(NKT):
                    nc.tensor.matmul(avs, lhsT=v1[:, kt, h, 0:33],
                                     rhs=e_sb[:, kt * QL:(kt + 1) * QL],
                                     start=(kt == 0), stop=(kt == NKT - 1))
                # denominator -> recip -> broadcast over 32 rows -> gating
                dn = smp.tile([33, QL], bf16, tag="dn")
                nc.vector.tensor_copy(dn[32:33, :], av[32:33, 0:QL])
                px = psX.tile([128, 512], f32, tag="px", name=f"denB{h}")
                nc.tensor.matmul(px[0:32, 0:QL], lhsT=ones33[32:33, :],
                                 rhs=dn[32:33, :], start=True, stop=True)
                recipB = smp.tile([32, QL], f32, tag="recipB")
                nc.vector.reciprocal_approx_fast(out=recipB[:], in_=px[0:32, 0:QL])
                t1 = smp.tile([32, QL], bf16, tag="t1")
                nc.vector.tensor_mul(t1[:], av[0:32, 0:QL], gT8[:, h, :])
                nc.vector.tensor_mul(go_all[0:32, h, :], t1[:], recipB[:])

            # ---- output projection ----
            for qt in range(2):
                qsl = slice(qt * 128, (qt + 1) * 128)
                pso = psX.tile([128, 512], f32, tag="px", name=f"pso{qt}")
                for h in range(H):
                    nc.tensor.matmul(pso[:, 0:128], lhsT=go_all[0:32, h, qsl],
                                     rhs=wo_sb[:, h, :], start=(h == 0), stop=False)
                nc.tensor.matmul(pso[:, 0:128], lhsT=ones_row[:, 0:128],
                                 rhs=bo_sb[:], start=False, stop=True)
                out_sb = smp.tile([128, 128], f32, tag="out")
                nc.vector.tensor_copy(out_sb[:], pso[:, 0:128])
                nc.sync.dma_start(
                    out.rearrange("(a p) c -> a p c", p=128)[qt], out_sb[:])

    nc.compile()
    return nc


def _get_nc():
    if "nc" not in _CACHE:
        _CACHE["nc"] = build_nc()
    return _CACHE["nc"]


def make_in_maps(q_x, kv_x, bias, distance, Wq, Wk, Wv, Wg, bg, Wo, bo):
    import ml_dtypes
    bf = ml_dtypes.bfloat16
    com = {
        "kv_x": np.ascontiguousarray(np.asarray(kv_x[0]).T).astype(bf),
        "Wq": (np.asarray(Wq) * SCALE).astype(bf),
        "Wk": np.asarray(Wk).astype(bf),
        "Wv": np.asarray(Wv).astype(bf),
        "Wg": np.asarray(Wg).astype(bf),
        "bg": np.asarray(bg).reshape(1, HD).astype(bf),
        "Wo": np.ascontiguousarray(
            np.asarray(Wo).reshape(H, 32, CQ).transpose(1, 0, 2)).astype(bf),
        "bo": np.asarray(bo).reshape(1, CQ).astype(bf),
    }
    maps = []
    for i in range(NCORES):
        s = slice(i * QL, (i + 1) * QL)
        m = dict(com)
        m["q_x"] = np.ascontiguousarray(np.asarray(q_x[0, s]).T).astype(bf)
        # bias[q,k] -> [p, kt*q] with k = kt*128 + p
        bslc = np.asarray(bias[0, 0, s])                              # [q, k]
        m["bias"] = np.ascontiguousarray(
            bslc.T.reshape(NKT, 128, QL).transpose(1, 0, 2).reshape(128, KW)
        ).astype(bf)
        # distance[q,k,h] -> [h, p, kt*q]
        dslc = np.asarray(distance[0, s])                             # [q, k, h]
        m["distance"] = np.ascontiguousarray(
            dslc.transpose(2, 1, 0).reshape(H, NKT, 128, QL)
                .transpose(0, 2, 1, 3).reshape(H, 128, KW)).astype(bf)
        maps.append(m)
    return maps


def kernel(q_x, kv_x, bias, distance, Wq, Wk, Wv, Wg, bg, Wo, bo, trace=False):
    from concourse.bass_utils import run_bass_kernel_spmd

    nc = _get_nc()
    in_maps = make_in_maps(q_x, kv_x, bias, distance, Wq, Wk, Wv, Wg, bg, Wo, bo)
    res = run_bass_kernel_spmd(nc, in_maps, core_ids=list(range(NCORES)),
                               trace=trace)
    _CACHE["last_result"] = res
    out = np.concatenate([res.results[i]["out"] for i in range(NCORES)], axis=0)
    return out.reshape(B, Q, CQ).astype(np.float32)


# revision 12
# speedup vs baseline: 1.4332x; 1.1078x over previous
"""Trainium2 Bass kernel for gated multi-head attention (nn_Attention_71751723647784).

Reference computation (B=1, Q=K=2048, CQ=CK=CV=128, H=8, CH=32, HD=256):
    q = (q_x @ Wq)/sqrt(CH); k = kv_x @ Wk; v = kv_x @ Wv           (per-head CH=32)
    a = softmax(q k^T + bias + distance.transpose(0,3,1,2), axis=-1)
    o = (a @ v) * sigmoid(q_x @ Wg + bg);  out = o @ Wo + bo

Sharding: rows of Q across the 8 cores (256 query rows per core); every input
byte read exactly once (bias is shared across heads).

v2 design (k-major scores):
  - Bulk inputs are cast/laid out in bf16 on the host: distance arrives as
    [H, 128p, 16kt*256q] (k = kt*128+p), bias as [128p, 16kt*256q], q_x/kv_x
    pre-transposed to [c, q]/[c, k], Wq pre-scaled by 1/sqrt(CH).  Halves DMA
    bytes and removes every on-chip cast / PE input transpose of v1.
  - Scores are built k-major (k on partitions): sc[k,q] = qk (PE matmul)
    + (bias+dist) where bias+dist is one DVE bf16 add (2x mode) merged into
    PSUM with one PE identity-matmul.  exp on ACT reads PSUM directly; no
    max-subtraction needed (scores are O(6), bf16/f32 range is ample).
  - Softmax normalisation is deferred past the AV matmul: v carries an
    appended ones-column, so row 32 of each head's AV output is the softmax
    denominator for free.  recip via reciprocal_approx_fast, broadcast over
    the 32 ch rows with a tiny PE outer-product, folded into the gating mults.
  - No e-transposes (33us of DMA_TRANSPOSE in v1) and no separate
    normalisation pass over e (12us of DVE in v1).
"""

import math
import numpy as np

B, Q, KS = 1, 2048, 2048
CQ = 128
H, CH = 8, 32
HD = H * CH  # 256
NCORES = 8
QL = Q // NCORES       # 256 query rows per core
NKT = 16               # k tiles of 128
KW = NKT * QL          # 4096 score elements per partition per head
SCALE = 1.0 / math.sqrt(CH)
# exp chunks in k-tiles: 6+6+4 tiles -> ACT FD 1536/1536/1024
CHUNKS = [(0, 6), (6, 6), (12, 4)]

_CACHE = {}


def build_nc():
    from concourse import bacc
    import concourse.tile as tile
    import concourse.mybir as mybir
    from concourse.masks import make_identity

    f32 = mybir.dt.float32
    bf16 = mybir.dt.bfloat16
    AF = mybir.ActivationFunctionType

    nc = bacc.Bacc("TRN2", target_bir_lowering=False, debug=False)

    dist = nc.dram_tensor("distance", (H, 128, KW), bf16, kind="ExternalInput").ap()
    biasT = nc.dram_tensor("bias", (128, KW), bf16, kind="ExternalInput").ap()
    qxT = nc.dram_tensor("q_x", (CQ, QL), bf16, kind="ExternalInput").ap()
    kvxT = nc.dram_tensor("kv_x", (CQ, KS), bf16, kind="ExternalInput").ap()
    Wq = nc.dram_tensor("Wq", (CQ, HD), bf16, kind="ExternalInput").ap()  # pre-scaled
    Wk = nc.dram_tensor("Wk", (CQ, HD), bf16, kind="ExternalInput").ap()
    Wv = nc.dram_tensor("Wv", (CQ, HD), bf16, kind="ExternalInput").ap()
    Wg = nc.dram_tensor("Wg", (CQ, HD), bf16, kind="ExternalInput").ap()
    bgr = nc.dram_tensor("bg", (1, HD), bf16, kind="ExternalInput").ap()
    Wo = nc.dram_tensor("Wo", (32, H, CQ), bf16, kind="ExternalInput").ap()
    bo = nc.dram_tensor("bo", (1, CQ), bf16, kind="ExternalInput").ap()
    out = nc.dram_tensor("out", (QL, CQ), f32, kind="ExternalOutput").ap()

    with tile.TileContext(nc) as tc:
        with (
            tc.tile_pool(name="const", bufs=1) as constp,
            tc.tile_pool(name="wts", bufs=1) as wtp,
            tc.tile_pool(name="proj", bufs=1) as projp,
            tc.tile_pool(name="dist", bufs=3) as distp,
            tc.tile_pool(name="bd", bufs=2) as bdp,
            tc.tile_pool(name="e", bufs=2) as ep,
            tc.tile_pool(name="small", bufs=3) as smp,
            tc.tile_pool(name="psSC", bufs=2, space="PSUM") as psSC,
            tc.tile_pool(name="psX", bufs=2, space="PSUM") as psX,
        ):
            # ---- constants ----
            ident_bf = constp.tile([128, 128], bf16)
            make_identity(nc, ident_bf[:])
            ones_row = constp.tile([1, QL], bf16)
            nc.gpsimd.memset(ones_row[:], 1.0)
            ones33 = constp.tile([33, 32], bf16)
            nc.gpsimd.memset(ones33[:], 1.0)

            # ---- weight / input loads (scalar HWDGE queue) ----
            wq_sb = wtp.tile([128, HD], bf16)
            wk_sb = wtp.tile([128, HD], bf16)
            wv_sb = wtp.tile([128, HD], bf16)
            wg_sb = wtp.tile([128, HD], bf16)
            wo_sb = wtp.tile([32, H, 128], bf16)
            bo_sb = wtp.tile([1, 128], bf16)
            bg_sb = wtp.tile([1, HD], bf16)
            qxT_sb = wtp.tile([128, QL], bf16)
            kvxT_sb = wtp.tile([128, KS], bf16)
            biasT_sb = wtp.tile([128, KW], bf16)
            nc.scalar.dma_start(qxT_sb[:], qxT)
            nc.scalar.dma_start(wg_sb[:], Wg)
            nc.scalar.dma_start(bg_sb[:], bgr)
            nc.scalar.dma_start(wq_sb[:], Wq)
            nc.scalar.dma_start(wk_sb[:], Wk)
            nc.scalar.dma_start(wv_sb[:], Wv)
            nc.scalar.dma_start(wo_sb[:], Wo)
            nc.scalar.dma_start(bo_sb[:], bo)
            nc.scalar.dma_start(kvxT_sb[:], kvxT)
            nc.scalar.dma_start(biasT_sb[:], biasT)

            # ---- projections ----
            # gating first (sigmoid's ACT table loads before exp's).
            # gT8[c, h, q] = sigmoid(Wg_h^T qxT + bg_h), per-head at rows 0-31
            gT8 = projp.tile([32, H, QL], bf16)
            psg1 = psSC.tile([128, 1536], f32, tag="sc", name="psg1")
            psg2 = psX.tile([128, 512], f32, tag="px", name="psg2")
            for h in range(H):
                dst = psg1[0:32, h * QL:(h + 1) * QL] if h < 6 else \
                      psg2[0:32, (h - 6) * QL:(h - 5) * QL]
                nc.tensor.matmul(dst, lhsT=wg_sb[:, 32 * h:32 * h + 32],
                                 rhs=qxT_sb[:], start=True, stop=False)
                nc.tensor.matmul(dst, lhsT=bg_sb[:, 32 * h:32 * h + 32],
                                 rhs=ones_row[:], start=False, stop=True)
            nc.scalar.activation(
                gT8[:, 0:6, :].rearrange("c a q -> c (a q)"),
                psg1[0:32, 0:6 * QL], AF.Sigmoid)
            nc.scalar.activation(
                gT8[:, 6:8, :].rearrange("c a q -> c (a q)"),
                psg2[0:32, 0:2 * QL], AF.Sigmoid)
            # force the exp table load now, during the DMA window
            je = smp.tile([128, 2], bf16, tag="junk")
            nc.scalar.activation(je[:], qxT_sb[:, 0:2], AF.Exp)

            # qT[g][hd-half, q] (Wq pre-scaled on host)
            qT = []
            for g in range(2):
                psq = psX.tile([128, 512], f32, tag="px", name=f"psq{g}")
                nc.tensor.matmul(psq[:, 0:QL], lhsT=wq_sb[:, g * 128:(g + 1) * 128],
                                 rhs=qxT_sb[:], start=True, stop=True)
                qt = projp.tile([128, QL], bf16, tag=f"qT{g}", name=f"qT{g}")
                nc.vector.tensor_copy(qt[:], psq[:, 0:QL])
                qT.append(qt)
            # kT[g][hd-half, k] full width
            kT = []
            for g in range(2):
                kt_ = projp.tile([128, KS], bf16, tag=f"kT{g}", name=f"kT{g}")
                for c in range(2):
                    psk = psSC.tile([128, 1536], f32, tag="sc", name=f"psk{g}{c}")
                    for j in range(2):
                        nc.tensor.matmul(
                            psk[:, j * 512:(j + 1) * 512],
                            lhsT=wk_sb[:, g * 128:(g + 1) * 128],
                            rhs=kvxT_sb[:, c * 1024 + j * 512:c * 1024 + (j + 1) * 512],
                            start=True, stop=True)
                    nc.vector.tensor_copy(kt_[:, c * 1024:(c + 1) * 1024],
                                          psk[:, 0:1024])
                kT.append(kt_)
            # v1[k, kt, h, 0:32]=v, [.., 32]=1.0 (denominator column)
            v1 = projp.tile([128, NKT, H, 36], bf16)
            nc.gpsimd.memset(v1[:, :, :, 32:33], 1.0)
            for kt in range(NKT):
                psv = psX.tile([128, 512], f32, tag="px", name=f"psv{kt}")
                nc.tensor.matmul(psv[:, 0:HD],
                                 lhsT=kvxT_sb[:, kt * 128:(kt + 1) * 128],
                                 rhs=wv_sb[:], start=True, stop=True)
                nc.vector.tensor_copy(
                    v1[:, kt, :, 0:32],
                    psv[:, 0:HD].rearrange("p (h c) -> p h c", h=H))

            # ---- main loop over heads ----
            go_all = projp.tile([128, H, QL], bf16)  # rows 0-31 live
            for h in range(H):
                g, hl = h // 4, h % 4
                d_sb = distp.tile([128, KW], bf16, tag="dist")
                nc.sync.dma_start(d_sb[:], dist[h])
                # e = exp(qk) * exp(bias+dist): keeps the PE out of the
                # bias/dist merge entirely (identity-matmul merge costs 33k
                # PE cycles; the PE clock throttles under load, so PE cycles
                # are the scarcest resource).  exp is exact; the product is
                # a DVE bf16 2x op.
                bd = bdp.tile([128, KW], bf16, tag="bd")
                nc.vector.tensor_add(bd[:], d_sb[:], biasT_sb[:])
                e2 = bdp.tile([128, KW], bf16, tag="e2")
                nc.scalar.activation(e2[:], bd[:], AF.Exp)
                e_sb = ep.tile([128, KW], bf16, tag="e")
                for (kt0, nkt) in CHUNKS:
                    sc = psSC.tile([128, 1536], f32, tag="sc")
                    for j in range(nkt):
                        kt = kt0 + j
                        nc.tensor.matmul(
                            sc[:, j * QL:(j + 1) * QL],
                            lhsT=kT[g][32 * hl:32 * hl + 32, kt * 128:(kt + 1) * 128],
                            rhs=qT[g][32 * hl:32 * hl + 32, :],
                            start=True, stop=True, tile_position=(32 * hl, 0))
                    e1 = smp.tile([128, 1536], bf16, tag="e1")
                    nc.scalar.activation(e1[:, 0:nkt * QL], sc[:, 0:nkt * QL],
                                         AF.Exp)
                    nc.vector.tensor_mul(
                        e_sb[:, kt0 * QL:(kt0 + nkt) * QL],
                        e1[:, 0:nkt * QL],
                        e2[:, kt0 * QL:(kt0 + nkt) * QL])
                # AV with ones-column: rows 0-31 data, row 32 = denominator
                av = psX.tile([128, 512], f32, tag="px", name=f"av{h}")
                avs = av[0:33, 0:QL]
                for kt in range(NKT):
                    nc.tensor.matmul(avs, lhsT=v1[:, kt, h, 0:33],
                                     rhs=e_sb[:, kt * QL:(kt + 1) * QL],
                                     start=(kt == 0), stop=(kt == NKT - 1))
                # denominator -> recip -> broadcast over 32 rows -> gating
                dn = smp.tile([33, QL], bf16, tag="dn")
                nc.vector.tensor_copy(dn[32:33, :], av[32:33, 0:QL])
                px = psX.tile([128, 512], f32, tag="px", name=f"denB{h}")
                nc.tensor.matmul(px[0:32, 0:QL], lhsT=ones33[32:33, :],
                                 rhs=dn[32:33, :], start=True, stop=True)
                recipB = smp.tile([32, QL], f32, tag="recipB")
                nc.vector.reciprocal_approx_fast(out=recipB[:], in_=px[0:32, 0:QL])
                t1 = smp.tile([32, QL], bf16, tag="t1")
                nc.vector.tensor_mul(t1[:], av[0:32, 0:QL], gT8[:, h, :])
                nc.vector.tensor_mul(go_all[0:32, h, :], t1[:], recipB[:])

            # ---- output projection ----
            for qt in range(2):
                qsl = slice(qt * 128, (qt + 1) * 128)
                pso = psX.tile([128, 512], f32, tag="px", name=f"pso{qt}")
                for h in range(H):
                    nc.tensor.matmul(pso[:, 0:128], lhsT=go_all[0:32, h, qsl],
                                     rhs=wo_sb[:, h, :], start=(h == 0), stop=False)
                nc.tensor.matmul(pso[:, 0:128], lhsT=ones_row[:, 0:128],
                                 rhs=bo_sb[:], start=False, stop=True)
                out_sb = smp.tile([128, 128], f32, tag="out")
                nc.vector.tensor_copy(out_sb[:], pso[:, 0:128])
                nc.sync.dma_start(
                    out.rearrange("(a p) c -> a p c", p=128)[qt], out_sb[:])

    nc.compile()
    return nc


def _get_nc():
    if "nc" not in _CACHE:
        _CACHE["nc"] = build_nc()
    return _CACHE["nc"]


def make_in_maps(q_x, kv_x, bias, distance, Wq, Wk, Wv, Wg, bg, Wo, bo):
    import ml_dtypes
    bf = ml_dtypes.bfloat16
    com = {
        "kv_x": np.ascontiguousarray(np.asarray(kv_x[0]).T).astype(bf),
        "Wq": (np.asarray(Wq) * SCALE).astype(bf),
        "Wk": np.asarray(Wk).astype(bf),
        "Wv": np.asarray(Wv).astype(bf),
        "Wg": np.asarray(Wg).astype(bf),
        "bg": np.asarray(bg).reshape(1, HD).astype(bf),
        "Wo": np.ascontiguousarray(
            np.asarray(Wo).reshape(H, 32, CQ).transpose(1, 0, 2)).astype(bf),
        "bo": np.asarray(bo).reshape(1, CQ).astype(bf),
    }
    maps = []
    for i in range(NCORES):
        s = slice(i * QL, (i + 1) * QL)
        m = dict(com)
        m["q_x"] = np.ascontiguousarray(np.asarray(q_x[0, s]).T).astype(bf)
        # bias[q,k] -> [p, kt*q] with k = kt*128 + p
        bslc = np.asarray(bias[0, 0, s])                              # [q, k]
        m["bias"] = np.ascontiguousarray(
            bslc.T.reshape(NKT, 128, QL).transpose(1, 0, 2).reshape(128, KW)
        ).astype(bf)
        # distance[q,k,h] -> [h, p, kt*q]
        dslc = np.asarray(distance[0, s])                             # [q, k, h]
        m["distance"] = np.ascontiguousarray(
            dslc.transpose(2, 1, 0).reshape(H, NKT, 128, QL)
                .transpose(0, 2, 1, 3).reshape(H, 128, KW)).astype(bf)
        maps.append(m)
    return maps


def kernel(q_x, kv_x, bias, distance, Wq, Wk, Wv, Wg, bg, Wo, bo, trace=False):
    from concourse.bass_utils import run_bass_kernel_spmd

    nc = _get_nc()
    in_maps = make_in_maps(q_x, kv_x, bias, distance, Wq, Wk, Wv, Wg, bg, Wo, bo)
    res = run_bass_kernel_spmd(nc, in_maps, core_ids=list(range(NCORES)),
                               trace=trace)
    _CACHE["last_result"] = res
    out = np.concatenate([res.results[i]["out"] for i in range(NCORES)], axis=0)
    return out.reshape(B, Q, CQ).astype(np.float32)


# revision 15
# speedup vs baseline: 1.5272x; 1.0656x over previous
"""Trainium2 Bass kernel for gated multi-head attention (nn_Attention_71751723647784).

Reference computation (B=1, Q=K=2048, CQ=CK=CV=128, H=8, CH=32, HD=256):
    q = (q_x @ Wq)/sqrt(CH); k = kv_x @ Wk; v = kv_x @ Wv           (per-head CH=32)
    a = softmax(q k^T + bias + distance.transpose(0,3,1,2), axis=-1)
    o = (a @ v) * sigmoid(q_x @ Wg + bg);  out = o @ Wo + bo

Sharding: rows of Q across the 8 cores (256 query rows per core); every input
byte read exactly once (bias is shared across heads).

v2 design (k-major scores):
  - Bulk inputs are cast/laid out in bf16 on the host: distance arrives as
    [H, 128p, 16kt*256q] (k = kt*128+p), bias as [128p, 16kt*256q], q_x/kv_x
    pre-transposed to [c, q]/[c, k], Wq pre-scaled by 1/sqrt(CH).  Halves DMA
    bytes and removes every on-chip cast / PE input transpose of v1.
  - Scores are built k-major (k on partitions): sc[k,q] = qk (PE matmul)
    + (bias+dist) where bias+dist is one DVE bf16 add (2x mode) merged into
    PSUM with one PE identity-matmul.  exp on ACT reads PSUM directly; no
    max-subtraction needed (scores are O(6), bf16/f32 range is ample).
  - Softmax normalisation is deferred past the AV matmul: v carries an
    appended ones-column, so row 32 of each head's AV output is the softmax
    denominator for free.  recip via reciprocal_approx_fast, broadcast over
    the 32 ch rows with a tiny PE outer-product, folded into the gating mults.
  - No e-transposes (33us of DMA_TRANSPOSE in v1) and no separate
    normalisation pass over e (12us of DVE in v1).
"""

import math
import numpy as np

B, Q, KS = 1, 2048, 2048
CQ = 128
H, CH = 8, 32
HD = H * CH  # 256
NCORES = 8
QL = Q // NCORES       # 256 query rows per core
NKT = 16               # k tiles of 128
KW = NKT * QL          # 4096 score elements per partition per head
SCALE = 1.0 / math.sqrt(CH)
# exp chunks in k-tiles: 6+6+4 tiles -> ACT FD 1536/1536/1024
CHUNKS = [(0, 6), (6, 6), (12, 4)]

_CACHE = {}


def build_nc():
    from concourse import bacc
    import concourse.tile as tile
    import concourse.mybir as mybir
    from concourse.masks import make_identity

    f32 = mybir.dt.float32
    bf16 = mybir.dt.bfloat16
    AF = mybir.ActivationFunctionType

    nc = bacc.Bacc("TRN2", target_bir_lowering=False, debug=False)

    dist = nc.dram_tensor("distance", (H, 128, KW), bf16, kind="ExternalInput").ap()
    biasT = nc.dram_tensor("bias", (128, KW), bf16, kind="ExternalInput").ap()
    qxT = nc.dram_tensor("q_x", (CQ, QL), bf16, kind="ExternalInput").ap()
    kvxT = nc.dram_tensor("kv_x", (CQ, KS), bf16, kind="ExternalInput").ap()
    Wq = nc.dram_tensor("Wq", (CQ, HD), bf16, kind="ExternalInput").ap()  # pre-scaled
    Wk = nc.dram_tensor("Wk", (CQ, HD), bf16, kind="ExternalInput").ap()
    Wv = nc.dram_tensor("Wv", (CQ, HD), bf16, kind="ExternalInput").ap()
    Wg = nc.dram_tensor("Wg", (CQ, HD), bf16, kind="ExternalInput").ap()
    bgr = nc.dram_tensor("bg", (1, HD), bf16, kind="ExternalInput").ap()
    Wo = nc.dram_tensor("Wo", (32, H, CQ), bf16, kind="ExternalInput").ap()
    bo = nc.dram_tensor("bo", (1, CQ), bf16, kind="ExternalInput").ap()
    out = nc.dram_tensor("out", (QL, CQ), f32, kind="ExternalOutput").ap()

    with tile.TileContext(nc) as tc:
        with (
            tc.tile_pool(name="const", bufs=1) as constp,
            tc.tile_pool(name="wts", bufs=1) as wtp,
            tc.tile_pool(name="proj", bufs=1) as projp,
            tc.tile_pool(name="dist", bufs=4) as distp,
            tc.tile_pool(name="bd", bufs=3) as bdp,
            tc.tile_pool(name="e", bufs=3) as ep,
            tc.tile_pool(name="small", bufs=3) as smp,
            tc.tile_pool(name="psSC", bufs=2, space="PSUM") as psSC,
            tc.tile_pool(name="psX", bufs=2, space="PSUM") as psX,
        ):
            # ---- constants ----
            ident_bf = constp.tile([128, 128], bf16)
            make_identity(nc, ident_bf[:])
            ones_row = constp.tile([1, QL], bf16)
            nc.gpsimd.memset(ones_row[:], 1.0)
            ones33 = constp.tile([33, 32], bf16)
            nc.gpsimd.memset(ones33[:], 1.0)

            # ---- weight / input loads (scalar HWDGE queue) ----
            wq_sb = wtp.tile([128, HD], bf16)
            wk_sb = wtp.tile([128, HD], bf16)
            wv_sb = wtp.tile([128, HD], bf16)
            wg_sb = wtp.tile([128, HD], bf16)
            wo_sb = wtp.tile([32, H, 128], bf16)
            bo_sb = wtp.tile([1, 128], bf16)
            bg_sb = wtp.tile([1, HD], bf16)
            qxT_sb = wtp.tile([128, QL], bf16)
            kvxT_sb = wtp.tile([128, KS], bf16)
            biasT_sb = wtp.tile([128, KW], bf16)
            nc.scalar.dma_start(qxT_sb[:], qxT)
            nc.scalar.dma_start(wg_sb[:], Wg)
            nc.scalar.dma_start(bg_sb[:], bgr)
            nc.scalar.dma_start(wq_sb[:], Wq)
            nc.scalar.dma_start(wk_sb[:], Wk)
            nc.scalar.dma_start(wv_sb[:], Wv)
            nc.scalar.dma_start(wo_sb[:], Wo)
            nc.scalar.dma_start(bo_sb[:], bo)
            nc.scalar.dma_start(kvxT_sb[:], kvxT)
            nc.scalar.dma_start(biasT_sb[:], biasT)

            # ---- projections ----
            # gating first (sigmoid's ACT table loads before exp's).
            # gT8[c, h, q] = sigmoid(Wg_h^T qxT + bg_h), per-head at rows 0-31
            gT8 = projp.tile([32, H, QL], bf16)
            psg1 = psSC.tile([128, 1536], f32, tag="sc", name="psg1")
            psg2 = psX.tile([128, 512], f32, tag="px", name="psg2")
            for h in range(H):
                dst = psg1[0:32, h * QL:(h + 1) * QL] if h < 6 else \
                      psg2[0:32, (h - 6) * QL:(h - 5) * QL]
                nc.tensor.matmul(dst, lhsT=wg_sb[:, 32 * h:32 * h + 32],
                                 rhs=qxT_sb[:], start=True, stop=False)
                nc.tensor.matmul(dst, lhsT=bg_sb[:, 32 * h:32 * h + 32],
                                 rhs=ones_row[:], start=False, stop=True)
            nc.scalar.activation(
                gT8[:, 0:6, :].rearrange("c a q -> c (a q)"),
                psg1[0:32, 0:6 * QL], AF.Sigmoid)
            nc.scalar.activation(
                gT8[:, 6:8, :].rearrange("c a q -> c (a q)"),
                psg2[0:32, 0:2 * QL], AF.Sigmoid)
            # force the exp table load now, during the DMA window
            je = smp.tile([128, 2], bf16, tag="junk")
            nc.scalar.activation(je[:], qxT_sb[:, 0:2], AF.Exp)

            # qT[g][hd-half, q] (Wq pre-scaled on host)
            qT = []
            for g in range(2):
                psq = psX.tile([128, 512], f32, tag="px", name=f"psq{g}")
                nc.tensor.matmul(psq[:, 0:QL], lhsT=wq_sb[:, g * 128:(g + 1) * 128],
                                 rhs=qxT_sb[:], start=True, stop=True)
                qt = projp.tile([128, QL], bf16, tag=f"qT{g}", name=f"qT{g}")
                nc.vector.tensor_copy(qt[:], psq[:, 0:QL])
                qT.append(qt)
            # kT[g][hd-half, k] full width
            kT = []
            for g in range(2):
                kt_ = projp.tile([128, KS], bf16, tag=f"kT{g}", name=f"kT{g}")
                for c in range(2):
                    psk = psSC.tile([128, 1536], f32, tag="sc", name=f"psk{g}{c}")
                    for j in range(2):
                        nc.tensor.matmul(
                            psk[:, j * 512:(j + 1) * 512],
                            lhsT=wk_sb[:, g * 128:(g + 1) * 128],
                            rhs=kvxT_sb[:, c * 1024 + j * 512:c * 1024 + (j + 1) * 512],
                            start=True, stop=True)
                    nc.vector.tensor_copy(kt_[:, c * 1024:(c + 1) * 1024],
                                          psk[:, 0:1024])
                kT.append(kt_)
            # v1[k, kt, h, 0:32]=v, [.., 32]=1.0 (denominator column)
            v1 = projp.tile([128, NKT, H, 36], bf16)
            nc.gpsimd.memset(v1[:, :, :, 32:33], 1.0)
            for kt2 in range(NKT // 2):
                psv = psX.tile([128, 512], f32, tag="px", name=f"psv{kt2}")
                for j in range(2):
                    nc.tensor.matmul(psv[:, j * HD:(j + 1) * HD],
                                     lhsT=kvxT_sb[:, (2 * kt2 + j) * 128:
                                                  (2 * kt2 + j + 1) * 128],
                                     rhs=wv_sb[:], start=True, stop=True)
                nc.vector.tensor_copy(
                    v1[:, 2 * kt2:2 * kt2 + 2, :, 0:32],
                    psv[:, 0:2 * HD].rearrange("p (a h c) -> p a h c", a=2, h=H))

            # ---- main loop over heads ----
            go_all = projp.tile([128, H, QL], bf16)  # rows 0-31 live
            for h in range(H):
                g, hl = h // 4, h % 4
                d_sb = distp.tile([128, KW], bf16, tag="dist")
                nc.sync.dma_start(d_sb[:], dist[h])
                # e = exp(qk) * exp(bias+dist): keeps the PE out of the
                # bias/dist merge entirely (identity-matmul merge costs 33k
                # PE cycles; the PE clock throttles under load, so PE cycles
                # are the scarcest resource).  exp is exact; the product is
                # a DVE bf16 2x op.
                bd = bdp.tile([128, KW], bf16, tag="bd")
                if h in (0, 4):
                    # GPSIMD is otherwise idle; let it carry one bd-add per
                    # half to take load off the DVE
                    nc.gpsimd.tensor_add(bd[:], d_sb[:], biasT_sb[:])
                else:
                    nc.vector.tensor_add(bd[:], d_sb[:], biasT_sb[:])
                e2 = bdp.tile([128, KW], bf16, tag="e2")
                nc.scalar.activation(e2[:], bd[:], AF.Exp)
                e_sb = ep.tile([128, KW], bf16, tag="e")
                for (kt0, nkt) in CHUNKS:
                    sc = psSC.tile([128, 1536], f32, tag="sc")
                    for j in range(nkt):
                        kt = kt0 + j
                        nc.tensor.matmul(
                            sc[:, j * QL:(j + 1) * QL],
                            lhsT=kT[g][32 * hl:32 * hl + 32, kt * 128:(kt + 1) * 128],
                            rhs=qT[g][32 * hl:32 * hl + 32, :],
                            start=True, stop=True, tile_position=(32 * hl, 0))
                    e1 = smp.tile([128, 1536], bf16, tag="e1")
                    nc.scalar.activation(e1[:, 0:nkt * QL], sc[:, 0:nkt * QL],
                                         AF.Exp)
                    nc.vector.tensor_mul(
                        e_sb[:, kt0 * QL:(kt0 + nkt) * QL],
                        e1[:, 0:nkt * QL],
                        e2[:, kt0 * QL:(kt0 + nkt) * QL])
                # AV with ones-column: rows 0-31 data, row 32 = denominator
                av = psX.tile([128, 512], f32, tag="px", name=f"av{h}")
                avs = av[0:33, 0:QL]
                for kt in range(NKT):
                    nc.tensor.matmul(avs, lhsT=v1[:, kt, h, 0:33],
                                     rhs=e_sb[:, kt * QL:(kt + 1) * QL],
                                     start=(kt == 0), stop=(kt == NKT - 1))
                # denominator -> recip -> broadcast over 32 rows -> gating
                dn = smp.tile([33, QL], bf16, tag="dn")
                nc.vector.tensor_copy(dn[32:33, :], av[32:33, 0:QL])
                px = psX.tile([128, 512], f32, tag="px", name=f"denB{h}")
                nc.tensor.matmul(px[0:32, 0:QL], lhsT=ones33[32:33, :],
                                 rhs=dn[32:33, :], start=True, stop=True)
                recipB = smp.tile([32, QL], f32, tag="recipB")
                nc.vector.reciprocal_approx_fast(out=recipB[:], in_=px[0:32, 0:QL])
                t1 = smp.tile([32, QL], bf16, tag="t1")
                nc.vector.tensor_mul(t1[:], av[0:32, 0:QL], gT8[:, h, :])
                nc.vector.tensor_mul(go_all[0:32, h, :], t1[:], recipB[:])

            # ---- output projection ----
            for qt in range(2):
                qsl = slice(qt * 128, (qt + 1) * 128)
                pso = psX.tile([128, 512], f32, tag="px", name=f"pso{qt}")
                for h in range(H):
                    nc.tensor.matmul(pso[:, 0:128], lhsT=go_all[0:32, h, qsl],
                                     rhs=wo_sb[:, h, :], start=(h == 0), stop=False)
                nc.tensor.matmul(pso[:, 0:128], lhsT=ones_row[:, 0:128],
                                 rhs=bo_sb[:], start=False, stop=True)
                out_sb = smp.tile([128, 128], f32, tag="out")
                nc.vector.tensor_copy(out_sb[:], pso[:, 0:128])
                nc.sync.dma_start(
                    out.rearrange("(a p) c -> a p c", p=128)[qt], out_sb[:])

    nc.compile()
    return nc


def _get_nc():
    if "nc" not in _CACHE:
        _CACHE["nc"] = build_nc()
    return _CACHE["nc"]


def make_in_maps(q_x, kv_x, bias, distance, Wq, Wk, Wv, Wg, bg, Wo, bo):
    import ml_dtypes
    bf = ml_dtypes.bfloat16
    com = {
        "kv_x": np.ascontiguousarray(np.asarray(kv_x[0]).T).astype(bf),
        "Wq": (np.asarray(Wq) * SCALE).astype(bf),
        "Wk": np.asarray(Wk).astype(bf),
        "Wv": np.asarray(Wv).astype(bf),
        "Wg": np.asarray(Wg).astype(bf),
        "bg": np.asarray(bg).reshape(1, HD).astype(bf),
        "Wo": np.ascontiguousarray(
            np.asarray(Wo).reshape(H, 32, CQ).transpose(1, 0, 2)).astype(bf),
        "bo": np.asarray(bo).reshape(1, CQ).astype(bf),
    }
    maps = []
    for i in range(NCORES):
        s = slice(i * QL, (i + 1) * QL)
        m = dict(com)
        m["q_x"] = np.ascontiguousarray(np.asarray(q_x[0, s]).T).astype(bf)
        # bias[q,k] -> [p, kt*q] with k = kt*128 + p
        bslc = np.asarray(bias[0, 0, s])                              # [q, k]
        m["bias"] = np.ascontiguousarray(
            bslc.T.reshape(NKT, 128, QL).transpose(1, 0, 2).reshape(128, KW)
        ).astype(bf)
        # distance[q,k,h] -> [h, p, kt*q]
        dslc = np.asarray(distance[0, s])                             # [q, k, h]
        m["distance"] = np.ascontiguousarray(
            dslc.transpose(2, 1, 0).reshape(H, NKT, 128, QL)
                .transpose(0, 2, 1, 3).reshape(H, 128, KW)).astype(bf)
        maps.append(m)
    return maps


def kernel(q_x, kv_x, bias, distance, Wq, Wk, Wv, Wg, bg, Wo, bo, trace=False):
    from concourse.bass_utils import run_bass_kernel_spmd

    nc = _get_nc()
    in_maps = make_in_maps(q_x, kv_x, bias, distance, Wq, Wk, Wv, Wg, bg, Wo, bo)
    res = run_bass_kernel_spmd(nc, in_maps, core_ids=list(range(NCORES)),
                               trace=trace)
    _CACHE["last_result"] = res
    out = np.concatenate([res.results[i]["out"] for i in range(NCORES)], axis=0)
    return out.reshape(B, Q, CQ).astype(np.float32)
